# revision 23
# baseline (speedup 1.0000x reference)
"""Trainium2 Bass kernel for nn_AttentionWithVQ (B=4, N=2048, DIM=512, H=8,
depthwise-conv positional term, softmax attention, output projection).

Sharding: data-parallel over B (4 batches x 2 core-groups) and tensor-parallel
over heads (4 heads per core) -> 8 cores, fully independent per core except a
final partial-sum over the two head-groups of each batch, done on host at
gather time (the output projection contracts over heads).

Core algorithmic fusion: the score matrix
    S = 0.5*(scale * q @ k^T + scale * conv1(m) @ conv2(s)^T)
is ONE matmul over a concatenated 128-feature axis:
    S = Qp^T @ Kp,  Qp = [q*scale*0.5 ; conv1(m)*scale*0.5], Kp = [k ; conv2(s)]
which exactly fills the 128x128 PE array contraction dim.

Softmax denominators: each V stationary block is padded to 128 columns with 64
REPLICATED ones-columns shared between head pairs (block layout
[v0|1*64|v1|v2|1*64|v3]; even head reads [v|1], odd head [1|v]), so attn@V
leaves the denominator already broadcast across 64 PSUM partitions on the
opposite half from the numerator, and the numerator half matches the aT
(projection-layout) rows for BOTH parities.  The drain is then pure on-chip
dataflow: fp32 staging copy, one SBUF->SBUF partition-shift DMA, an in-place
fast-approx reciprocal, and one multiply that writes normalized output
STRAIGHT into aT (no DRAM bounce, no output shift DMAs).

Schedule: the kernel is paced by the Scalar engine's 128 exp() instructions
(~1.11us each).  Everything else is arranged around keeping that stream dense:
  - inputs are packed host-side so every DMA is one large contiguous block
    (strided HBM reads are descriptor-dominated); wave 1 carries only the
    bytes the first attention block needs, wave 2 (queue-FIFO gated) streams
    the rest behind it.  A dummy exp() preloads the ACT table during the DMAs.
  - v-projection, remaining qkv chunks, t=1 convs and the previous stripe's
    output projection are emitted as PE/DVE fillers inside the attention loop.
  - loop order stripe-outer/head-inner so each stripe's projection + output
    DMA overlaps the next stripe's attention.
"""


import sys

sys.path.insert(0, "/opt/trn_rl_repo")

import numpy as np

# ---------------------------------------------------------------- constants
B, N, DIM, HEAD, VQE_K = 4, 2048, 512, 8, 3
Dh = DIM // HEAD            # 64
HPC = HEAD // 2             # heads per core (8 cores = 4 batch * 2 groups)
P = 128
NKB = N // P                # 16 key blocks
FB = 512                    # one fp32 PSUM bank
FBS = 1024                  # attention stripe chunk (2 banks)
NST = N // FBS              # 2 q-stripes
SCALE_Q = Dh ** -0.5 * 0.5  # folds the 0.5 score scale into the q/conv1 side
VW = 384                    # v_sb block pitch: [v0|1*64|v1|v2|1*64|v3]
VOFF = (0, 64, 192, 256)    # per-head stationary column offset in a v block
CA = 1032                   # conv-input wave-1 column split (1024 + halo pad)

_DEFAULT_CFG = {}
_CACHE = {}


# ---------------------------------------------------------------- host prep
def _host_prep(core, inp):
    """Build the per-core input arrays (sharding + layout permutations).

    Every array is packed so the device-side DMA reads one large contiguous
    block per destination region (strided HBM reads are descriptor-dominated).
    """
    import ml_dtypes

    bf16 = ml_dtypes.bfloat16
    b, g = core // 2, core % 2
    f32 = np.float32
    x, m, s = inp["x"], inp["m"], inp["s"]
    qkv_w, qkv_b = inp["qkv_w"], inp["qkv_b"]
    proj_w = inp["proj_w"]
    p1w = inp["pe1_w"].reshape(HEAD, VQE_K)
    p2w = inp["pe2_w"].reshape(HEAD, VQE_K)
    pe1_b, pe2_b = inp["pe1_b"], inp["pe2_b"]

    d = {}
    xt = np.ascontiguousarray(x[b].T).astype(bf16)       # [512, 2048]
    # xta/xtb: [128, 4*w] partition-major, c-tiles side by side.  Wave-2
    # transfers re-write a 16-column overlap of their wave-1 predecessor
    # (identical data) purely to create a write-after-write dependency that
    # orders wave 2 behind wave 1 (the tile scheduler ignores queue order).
    xta = np.empty((128, 4, 1024), bf16)
    xtb = np.empty((128, 4, 1040), bf16)
    for c in range(4):
        xta[:, c, :] = xt[c * 128:(c + 1) * 128, 0:1024]
        xtb[:, c, :] = xt[c * 128:(c + 1) * 128, 1008:2048]
    d["xta"] = xta.reshape(128, 4096)
    d["xtb"] = xtb.reshape(128, 4160)

    # m/s transposed, tile t rows = [head(2t+1) feats ; head(2t) feats]
    mt = np.empty((256, N), f32)
    st = np.empty((256, N), f32)
    mcw = np.zeros((128, 8), f32)
    scw = np.zeros((128, 8), f32)
    for t in range(2):
        h_lo, h_hi = g * 4 + 2 * t + 1, g * 4 + 2 * t
        mt[t * 128:t * 128 + 64] = m[b][:, h_lo * 64:(h_lo + 1) * 64].T
        mt[t * 128 + 64:t * 128 + 128] = m[b][:, h_hi * 64:(h_hi + 1) * 64].T
        st[t * 128:t * 128 + 64] = s[b][:, h_lo * 64:(h_lo + 1) * 64].T
        st[t * 128 + 64:t * 128 + 128] = s[b][:, h_hi * 64:(h_hi + 1) * 64].T
        for p in range(128):
            h = g * 4 + 2 * t + (1 if p < 64 else 0)
            mcw[p, 4 * t:4 * t + 3] = p1w[h] * SCALE_Q
            scw[p, 4 * t:4 * t + 3] = p2w[h]
            mcw[p, 4 * t + 3] = pe1_b[h] * SCALE_Q
            scw[p, 4 * t + 3] = pe2_b[h]
    mtc, stc = mt.astype(bf16), st.astype(bf16)
    # conv inputs live in one [128, 4096] tile per source (t0 | t1); the
    # wave-2 pieces overlap their predecessor by 16 cols (WAW chaining)
    d["mta"] = np.ascontiguousarray(mtc[0:128, 0:CA])
    d["mtb"] = np.ascontiguousarray(mtc[0:128, CA - 16:N])
    d["sta"] = np.ascontiguousarray(stc[0:128, 0:CA])
    d["stb"] = np.ascontiguousarray(stc[0:128, CA - 16:N])
    d["mt1"] = np.ascontiguousarray(
        np.concatenate([mtc[0:128, N - 16:N], mtc[128:256, :]], axis=1))
    d["st1"] = np.ascontiguousarray(
        np.concatenate([stc[0:128, N - 16:N], stc[128:256, :]], axis=1))
    d["mcw"], d["scw"] = mcw, scw

    # q/k projection weights: chunk ch=(t, q|k) = [even-head rows; odd-head rows]
    wqk_f = np.empty((512, DIM), f32)
    qkb = np.zeros((128, 4), f32)
    for t in range(2):
        for j in range(2):  # 0=q, 1=k
            ch = 2 * t + j
            h_e, h_o = g * 4 + 2 * t, g * 4 + 2 * t + 1
            base = j * DIM
            wqk_f[ch * 128:ch * 128 + 64] = qkv_w[base + h_e * 64:base + (h_e + 1) * 64]
            wqk_f[ch * 128 + 64:(ch + 1) * 128] = qkv_w[base + h_o * 64:base + (h_o + 1) * 64]
            qkb[0:64, ch] = qkv_b[base + h_e * 64:base + (h_e + 1) * 64]
            qkb[64:128, ch] = qkv_b[base + h_o * 64:base + (h_o + 1) * 64]
            if j == 0:
                wqk_f[ch * 128:(ch + 1) * 128] *= SCALE_Q
                qkb[:, ch] *= SCALE_Q
    wqk_t = np.ascontiguousarray(wqk_f.T).astype(bf16)   # [c=512, f=512]
    wqka = np.empty((128, 4, 256), bf16)                 # ch0/ch1 (t=0 q,k)
    wqkb = np.empty((128, 4, 272), bf16)                 # ch2/ch3 + overlap
    for c in range(4):
        wqka[:, c, :] = wqk_t[c * 128:(c + 1) * 128, 0:256]
        wqkb[:, c, :] = wqk_t[c * 128:(c + 1) * 128, 240:512]
    d["wqka"] = wqka.reshape(128, 1024)
    d["wqkb"] = wqkb.reshape(128, 1088)
    d["qkb"] = qkb

    wv_t = np.ascontiguousarray(
        qkv_w[2 * DIM + g * 256:2 * DIM + (g + 1) * 256].T).astype(bf16)
    wvp = np.empty((128, 4, 256), bf16)
    for c in range(4):
        wvp[:, c, :] = wv_t[c * 128:(c + 1) * 128, :]
    d["wv"] = wvp.reshape(128, 1024)
    # v bias replicated along partitions: column order matches wv columns
    vb = qkv_b[2 * DIM + g * 256:2 * DIM + (g + 1) * 256]
    d["vbrep"] = np.broadcast_to(vb, (128, 256)).astype(bf16).copy()

    # proj rows in aT partition order: aT tile t partition p -> head
    # 2t+(p>=64), d=p%64.  Packed partition-major behind a 16-col duplicate
    # of the wqk tail so the transfer WAW-chains into the wqk tile region.
    pjt = np.empty((256, DIM), f32)
    for t in range(2):
        for p in range(128):
            h_l = 2 * t + (1 if p >= 64 else 0)
            h = g * 4 + h_l
            pjt[t * 128 + p] = proj_w[:, h * 64 + (p % 64)]
    pjtc = pjt.astype(bf16)
    pjtp = np.empty((128, 1040), bf16)
    pjtp[:, 0:16] = wqk_t[384:512, 496:512]
    for f in range(2):
        pjtp[:, 16 + f * 512:16 + (f + 1) * 512] = pjtc[f * 128:(f + 1) * 128]
    d["pjtp"] = pjtp
    return d


# ------------------------------------------------------------- device build
def _emit(tc, nc, io):
    from contextlib import ExitStack

    from concourse import mybir

    dt = mybir.dt
    f32 = dt.float32
    bf16 = dt.bfloat16
    AF = mybir.ActivationFunctionType
    ALU = mybir.AluOpType

    with ExitStack() as ctx:
        persist = ctx.enter_context(tc.tile_pool(name="persist", bufs=1))
        xtp = ctx.enter_context(tc.tile_pool(name="xtp", bufs=1))
        convp = ctx.enter_context(tc.tile_pool(name="convp", bufs=2))
        convyp = ctx.enter_context(tc.tile_pool(name="convyp", bufs=2))
        # PSUM: s_pool 2x2 banks, o_pool 1x2 banks, shp 2x1 bank = 8 banks
        s_pool = ctx.enter_context(
            tc.tile_pool(name="s_pool", bufs=2, space="PSUM"))
        o_pool = ctx.enter_context(
            tc.tile_pool(name="o_pool", bufs=1, space="PSUM"))
        shp = ctx.enter_context(tc.tile_pool(name="shp", bufs=2, space="PSUM"))
        esb = ctx.enter_context(tc.tile_pool(name="esb", bufs=8))
        stgp = ctx.enter_context(tc.tile_pool(name="stgp", bufs=2))
        bcp = ctx.enter_context(tc.tile_pool(name="bcp", bufs=2))
        obp = ctx.enter_context(tc.tile_pool(name="obp", bufs=3))

        # ---- persistent tiles (pjt lives in the wqk tile's tail so its
        # wave-2 transfer can WAW-chain behind the wqkb transfer)
        wqk_all = persist.tile([128, 3072], bf16, name="wqk", tag="wqk")
        wqk_sb = [wqk_all[:, c * 512:(c + 1) * 512] for c in range(4)]
        pjt_sb = [wqk_all[:, 2048 + f * 512:2048 + (f + 1) * 512]
                  for f in range(2)]
        wv_all = persist.tile([128, 1024], bf16, name="wv", tag="wv")
        wv_sb = [wv_all[:, c * 256:(c + 1) * 256] for c in range(4)]
        mcw_sb = persist.tile([128, 8], f32, name="mcw", tag="mcw")
        scw_sb = persist.tile([128, 8], f32, name="scw", tag="scw")
        qkb_sb = persist.tile([128, 4], f32, name="qkb", tag="qkb")
        vbr_sb = persist.tile([128, 256], bf16, name="vbrep", tag="vbrep")
        QP = [persist.tile([128, N], bf16, name=f"QP{h}", tag=f"QP{h}")
              for h in range(HPC)]
        KP = [persist.tile([128, N], bf16, name=f"KP{h}", tag=f"KP{h}")
              for h in range(HPC)]
        # one big V tile: 16 blocks at VW pitch [v0|1*64|v1|v2|1*64|v3], with
        # 64 cols of tail padding so the strided odd-head write AP stays in
        # bounds.  The ones-columns sit at 64 + k*192 for k=0..31, written by
        # ONE strided memset.
        v_big = persist.tile([128, NKB * VW + 128], bf16, name="vbig",
                             tag="vbig")
        aT = [persist.tile([128, N], bf16, name=f"aT{t}", tag=f"aT{t}")
              for t in range(2)]
        xt_all = xtp.tile([128, 4 * N], bf16, name="xt", tag="xt")
        xt_sb = [xt_all[:, c * N:(c + 1) * N] for c in range(4)]

        # conv inputs: one [128, 4096] tile per source, t0 | t1 halves
        cinS = {src: convp.tile([128, 2 * N], bf16, name=f"ci_{src}",
                                tag="cin") for src in ("st", "mt")}
        cin0 = {src: cinS[src][:, 0:N] for src in ("st", "mt")}
        cin1 = {src: cinS[src][:, N:2 * N] for src in ("st", "mt")}

        # ---- input DMAs.  Wave 1 = only what the first attention block
        # needs, every transfer one large contiguous HBM block.  The ACT
        # exp-table preload rides the scalar queue behind the tiny tiles.
        nc.scalar.dma_start(mcw_sb[:], io["mcw"][:, :])
        nc.scalar.dma_start(scw_sb[:], io["scw"][:, :])
        nc.scalar.dma_start(qkb_sb[:], io["qkb"][:, :])
        dum = persist.tile([1, 16], f32, name="dum", tag="dum")
        nc.vector.memset(dum[:], 0.0)
        nc.scalar.activation(dum[:], dum[:], AF.Exp)
        # warm-up matmuls: ~4us of PE activity trips the HAM clock gate to
        # full rate before the projection chunks need it
        warm = persist.tile([128, 512], bf16, name="warm", tag="warm")
        nc.vector.memset(warm[:], 0.25)
        wps = shp.tile([128, FB], f32, name="wps", tag="sh")
        for _ in range(10):
            nc.tensor.matmul(wps[:], warm[:, 0:128], warm[:],
                             start=True, stop=True, skip_group_check=True)
        # the ones-columns of every v block, one strided memset
        nc.vector.memset(
            v_big[:, 64:64 + 32 * 192].rearrange("p (s r) -> p s r",
                                                 r=192)[:, :, 0:64], 1.0)

        # xta in two halves so the first qkv chunks start at half-arrival
        xtv = xt_all.rearrange("p (c n) -> p c n", c=4)
        xav = io["xta"].rearrange("p (c n) -> p c n", c=4)
        nc.sync.dma_start(xtv[:, 0:2, 0:1024], xav[:, 0:2, :])
        nc.sync.dma_start(cin0["mt"][:, 0:CA], io["mta"][:, :])
        nc.sync.dma_start(xtv[:, 2:4, 0:1024], xav[:, 2:4, :])
        nc.sync.dma_start(cin0["st"][:, 0:CA], io["sta"][:, :])
        nc.gpsimd.dma_start(
            wqk_all[:, 0:2048].rearrange("p (c f) -> p c f", c=4)[:, :, 0:256],
            io["wqka"].rearrange("p (c f) -> p c f", c=4))
        nc.gpsimd.dma_start(
            wv_all.rearrange("p (c f) -> p c f", c=4),
            io["wv"].rearrange("p (c f) -> p c f", c=4))
        nc.gpsimd.dma_start(vbr_sb[:], io["vbrep"][:, :])

        # ---- helpers -----------------------------------------------------
        convy = {}

        def conv_ops(src, wv_, dst, t, xin, c0=0, c1=N):
            """Depthwise 3-tap conv for columns [c0,c1) of tile t of m/s.
            Column-ranged so the first chunk (which gates the first attention
            iteration) finishes early."""
            key = (src, t)
            if key not in convy:
                convy[key] = convyp.tile([128, N], bf16, name=f"cy_{src}{t}",
                                         tag="cy")
            y = convy[key]
            w0, w1, w2, cb = (wv_[:, 4 * t + k:4 * t + k + 1] for k in range(4))
            lo = max(c0, 1)
            hi = min(c1, N - 1)
            nc.vector.tensor_scalar(y[:, c0:c1], xin[:, c0:c1], w1, cb,
                                    ALU.mult, ALU.add)
            nc.vector.scalar_tensor_tensor(
                y[:, lo:c1], xin[:, lo - 1:c1 - 1], w0, y[:, lo:c1],
                ALU.mult, ALU.add)
            nc.vector.scalar_tensor_tensor(
                y[:, c0:hi], xin[:, c0 + 1:hi + 1], w2, y[:, c0:hi],
                ALU.mult, ALU.add)
            nc.vector.tensor_copy(dst[2 * t + 1][0:64, c0:c1], y[0:64, c0:c1])
            nc.vector.tensor_copy(dst[2 * t][64:128, c0:c1], y[64:128, c0:c1])

        def qkv_chunk(ch, qs, pool, tag, width):
            """q/k projection chunk ch over q-columns qs (width cols)."""
            for step in qkv_chunk_steps(ch, qs, pool, tag, width):
                step()

        def qkv_chunk_steps(ch, qs, pool, tag, width):
            """Same, but as a list of single-matmul emission steps so the
            chunk can be spread across attention iterations."""
            t, j = ch // 2, ch % 2
            dst = QP if j == 0 else KP
            nh = width // FB
            state = {}

            def mk(ih, c):
                def step():
                    if "ps" not in state:
                        state["ps"] = pool.tile([128, width], f32,
                                                name="psqk", tag=tag)
                    ps = state["ps"]
                    nc.tensor.matmul(
                        ps[:, ih * FB:(ih + 1) * FB],
                        wqk_sb[c][:, ch * 128:(ch + 1) * 128],
                        xt_sb[c][:, qs.start + ih * FB:qs.start + (ih + 1) * FB],
                        start=(c == 0), stop=(c == 3))
                    if ih == nh - 1 and c == 3:
                        nc.vector.tensor_scalar_add(
                            dst[2 * t][0:64, qs], ps[0:64, :],
                            qkb_sb[0:64, ch:ch + 1])
                        nc.vector.tensor_scalar_add(
                            dst[2 * t + 1][64:128, qs], ps[64:128, :],
                            qkb_sb[64:128, ch:ch + 1])
                return step

            return [mk(ih, c) for ih in range(nh) for c in range(4)]

        def v_block(blk):
            """v projection for key-block blk + bias into the [v|1]/[1|v]
            slots (even heads at cols {0,192}, odd heads at {128,320})."""
            bs = slice(blk * 128, (blk + 1) * 128)
            ps = shp.tile([128, 512], f32, name="psv", tag="sh")
            for c in range(4):
                nc.tensor.matmul(ps[:, 0:256], xt_sb[c][:, bs], wv_sb[c][:],
                                 start=(c == 0), stop=(c == 3))
            ps4 = ps[:, 0:256].rearrange("p (s r) -> p s r", s=2)
            vb4 = vbr_sb.rearrange("p (s r) -> p s r", s=2)
            dste = v_big[:, blk * VW:blk * VW + 384].rearrange(
                "p (s r) -> p s r", r=192)[:, :, 0:64]
            dsto = v_big[:, blk * VW + 128:blk * VW + 512].rearrange(
                "p (s r) -> p s r", r=192)[:, :, 0:64]
            nc.vector.scalar_tensor_tensor(
                dste, ps4[:, :, 0:64], 1.0, vb4[:, :, 0:64],
                ALU.mult, ALU.add)
            nc.vector.scalar_tensor_tensor(
                dsto, ps4[:, :, 64:128], 1.0, vb4[:, :, 64:128],
                ALU.mult, ALU.add)

        def drain(h, q2, o_ps, mul_eng=None, dma_q=None, last=False):
            """Normalize o_ps by the softmax denominators into aT.

            One fp32 staging copy frees o_ps; the denominator (already
            replicated across 64 partitions by the ones-columns of V) is
            partition-shifted to the numerator's half by a single SBUF->SBUF
            DMA, approx-reciprocal'd in place, and multiplied straight into
            aT (both parities lane-aligned)."""
            t, odd = h // 2, h % 2
            if mul_eng is None:
                mul_eng = nc.gpsimd
            dq = dma_q if dma_q is not None else nc.sync
            rows = slice(64, 128) if odd else slice(0, 64)
            drows = slice(0, 64) if odd else slice(64, 128)
            row = h * NST + q2
            stg = stgp.tile([128, FBS], f32, name=f"stg{row}", tag="stg")
            bc = bcp.tile([128, 2 * FBS], f32, name=f"bc{row}", tag="bc")
            cs = slice(q2 * FBS, (q2 + 1) * FBS)
            # single PSUM read frees o_ps for the next accumulation.  The
            # custom-DVE reciprocal only works at partition base 0, so the
            # partition-shift DMA runs before it (even heads: den lives at
            # 64:128) or after it (odd heads: den already at 0:64).  For the
            # last (odd) drain, the reciprocal reads the denominator straight
            # from PSUM so it starts before the staging copy finishes.
            if last and odd:
                # no staging at all: reciprocal straight off PSUM, bf16-cast
                # partition-shift on the SWDGE ring, multiply off PSUM
                bcl = stgp.tile([128, FBS], bf16, name="bcl", tag="stg")
                nc.vector.reciprocal_approx_fast(bc[0:64, 0:FBS],
                                                 o_ps[0:64, :])
                nc.gpsimd.dma_start(bcl[64:128, :], bc[0:64, 0:FBS])
                mul_eng.tensor_mul(aT[t][rows, cs], o_ps[rows, :],
                                   bcl[rows, :])
                return
            if odd:
                nc.vector.tensor_copy(stg[:], o_ps[:])
                nc.vector.reciprocal_approx_fast(bc[0:64, 0:FBS],
                                                 stg[0:64, :])
                dq.dma_start(bc[64:128, FBS:2 * FBS], bc[0:64, 0:FBS])
            else:
                nc.vector.tensor_copy(stg[:], o_ps[:])
                dq.dma_start(bc[0:64, 0:FBS], stg[64:128, :])
                nc.vector.reciprocal_approx_fast(bc[0:64, FBS:2 * FBS],
                                                 bc[0:64, 0:FBS])
            mul_eng.tensor_mul(aT[t][rows, cs], stg[rows, :],
                               bc[rows, FBS:2 * FBS])

        def proj_block_steps(blk):
            """Output projection for 128 q rows + bf16 store, as 2 steps."""
            bs = slice(blk * 128, (blk + 1) * 128)
            state = {}

            def s0():
                state["pj"] = shp.tile([128, FB], f32, name="pj", tag="sh")
                nc.tensor.matmul(state["pj"][:], aT[0][:, bs], pjt_sb[0][:],
                                 start=True, stop=False)

            def s1():
                pj = state["pj"]
                nc.tensor.matmul(pj[:], aT[1][:, bs], pjt_sb[1][:],
                                 start=False, stop=True)
                ob = obp.tile([128, FB], bf16, name="ob", tag="ob")
                nc.vector.tensor_copy(ob[:], pj[:])
                nc.sync.dma_start(io["out"][bs, :], ob[:])

            return [s0, s1]

        def proj_block(blk):
            for s in proj_block_steps(blk):
                s()

        # ---- prologue compute: t=0 convs (first chunks early, distinct
        # PSUM slots per qkv chunk so none waits on another's bias), the qkv
        # chunks head 0 stripe 0 needs; everything else is emitted as
        # fillers inside the attention loop.
        conv_ops("mt", mcw_sb, QP, 0, cin0["mt"], 0, 1024)
        conv_ops("st", scw_sb, KP, 0, cin0["st"], 0, 256)
        qkv_chunk(1, slice(0, 512), s_pool, "sps", 512)      # k(t0) 0:512
        qkv_chunk(1, slice(512, 1024), o_pool, "ops", 512)   # k(t0) 512:1024
        qkv_chunk(0, slice(0, 512), s_pool, "sps", 512)      # q(t0) 0:512
        qkv_chunk(0, slice(512, 1024), shp, "sh", 512)       # q(t0) 512:1024
        conv_ops("st", scw_sb, KP, 0, cin0["st"], 256, 1024)

        # ---- wave 2 DMAs.  Each transfer re-writes a 16-col overlap of its
        # wave-1 (or earlier wave-2) predecessor with identical data: the
        # write-after-write dependency orders it behind the predecessor's
        # completion (the tile scheduler reorders queues, so FIFO gating
        # alone cannot hold wave 2 back).
        nc.gpsimd.dma_start(
            xt_all.rearrange("p (c n) -> p c n", c=4)[:, :, 1008:2048],
            io["xtb"].rearrange("p (c n) -> p c n", c=4))
        nc.gpsimd.dma_start(cinS["st"][:, CA - 16:N], io["stb"][:, :])
        nc.gpsimd.dma_start(cinS["st"][:, N - 16:2 * N], io["st1"][:, :])
        nc.gpsimd.dma_start(cinS["mt"][:, CA - 16:N], io["mtb"][:, :])
        nc.gpsimd.dma_start(cinS["mt"][:, N - 16:2 * N], io["mt1"][:, :])
        nc.gpsimd.dma_start(
            wqk_all[:, 0:2048].rearrange("p (c f) -> p c f",
                                         c=4)[:, :, 240:512],
            io["wqkb"].rearrange("p (c f) -> p c f", c=4))
        nc.gpsimd.dma_start(wqk_all[:, 2032:3072], io["pjtp"][:, :])
        conv_ops("st", scw_sb, KP, 0, cin0["st"], 1024, 2048)

        # ---- attention: stripe-outer, head-inner, exp-paced.  Fillers are
        # single-matmul-sized emission steps, one consumed per nk iteration.
        def fillers_for(h, q2):
            # an entry may be a list of sub-steps (all emitted in one slot)
            fl = []
            if q2 == 0 and h == 0:
                # v blocks ride just-in-time: with the skewed loop, aV(k) is
                # emitted at iter k+1, so v(k) sits at slot k
                for blk in range(NKB):
                    fl.append(lambda b_=blk: v_block(b_))
                # k(t0) cols 1024:2048 in two 512-chunks; their xt columns
                # arrive with the second DMA wave
                qa = qkv_chunk_steps(1, slice(1024, 1536), shp, "sh", 512)
                qb = qkv_chunk_steps(1, slice(1536, 2048), shp, "sh", 512)
                for i, s in enumerate(qa):
                    fl[2 + i] = [fl[2 + i], s]
                for i, s in enumerate(qb):
                    fl[7 + i] = [fl[7 + i], s]
                fl[15] = [fl[15],
                          lambda: conv_ops("st", scw_sb, KP, 1, cin1["st"])]
            elif q2 == 0 and h == 1:
                # k(t1)+q(t1) stripe-0 columns and the t=1 q-side conv
                # (stripe-0 half) — all due by h2 iter 0
                for ch, qb in ((3, 0), (2, 0), (2, 1)):
                    fl += qkv_chunk_steps(ch, slice(qb * 512, (qb + 1) * 512),
                                          shp, "sh", 512)
                fl.insert(6, lambda: conv_ops("mt", mcw_sb, QP, 1,
                                              cin1["mt"], 0, 1024))
                fl.append(lambda: conv_ops("mt", mcw_sb, QP, 0, cin0["mt"],
                                           1024, N))
            elif q2 == 0 and h == 2:
                # k(t1) remaining columns (due by h2 iters 4/8/12) + the
                # stripe-1 half of the t=1 q-side conv
                for ch, qb in ((3, 1), (3, 2), (3, 3)):
                    fl += qkv_chunk_steps(ch, slice(qb * 512, (qb + 1) * 512),
                                          shp, "sh", 512)
                fl.append(lambda: conv_ops("mt", mcw_sb, QP, 1, cin1["mt"],
                                           1024, N))
            elif q2 == 0 and h == 3:
                # q(t0)/q(t1) stripe-1 columns (due by stripe 1)
                for ch, qb in ((0, 2), (0, 3), (2, 2), (2, 3)):
                    fl += qkv_chunk_steps(ch, slice(qb * 512, (qb + 1) * 512),
                                          shp, "sh", 512)
            elif q2 == 1 and h == 0:
                # previous stripe's projection; pad the first slots so the
                # PE never head-of-line blocks on the preceding drain
                fl += [None] * 6
                fl += proj_block_steps(0)
                fl += proj_block_steps(1)
            elif q2 == 1 and h == 2:
                for blk in range(2, 6):
                    fl += proj_block_steps(blk)
            elif q2 == 1 and h == 1:
                fl += proj_block_steps(6)
                fl += proj_block_steps(7)
            return fl

        for q2 in range(NST):
            for h in ((0, 1, 2, 3) if q2 == 0 else (0, 2, 1, 3)):
                voff = VOFF[h]
                cs0 = q2 * FBS
                fl = fillers_for(h, q2)
                o_ps = o_pool.tile([128, FBS], f32, name=f"o{h}_{q2}",
                                   tag="ops")
                # skewed pipeline: scores run one iteration ahead of
                # exp/attnV, so the first attnV's wait on the previous
                # head's staging copy hides behind already-queued scores
                def exp_av(nk, s_prev):
                    e = esb.tile([128, FBS], bf16, name="e", tag="e")
                    nc.scalar.activation(e[:], s_prev[:], AF.Exp)
                    vst = v_big[:, nk * VW + voff:nk * VW + voff + 128]
                    for ih in range(2):
                        nc.tensor.matmul(
                            o_ps[:, ih * FB:(ih + 1) * FB], vst,
                            e[:, ih * FB:(ih + 1) * FB],
                            start=(nk == 0), stop=(nk == NKB - 1))

                s_prev = None
                for nk in range(NKB):
                    if fl:
                        f = fl.pop(0)
                        for g in (f if isinstance(f, list) else [f]):
                            if g is not None:
                                g()
                    ks = slice(nk * 128, (nk + 1) * 128)
                    s_ps = s_pool.tile([128, FBS], f32, name="sps", tag="sps")
                    for ih in range(2):
                        nc.tensor.matmul(
                            s_ps[:, ih * FB:(ih + 1) * FB], KP[h][:, ks],
                            QP[h][:, cs0 + ih * FB:cs0 + (ih + 1) * FB],
                            start=True, stop=True)
                    if s_prev is not None:
                        exp_av(nk - 1, s_prev)
                    s_prev = s_ps
                exp_av(NKB - 1, s_prev)
                for f in fl:
                    for g in (f if isinstance(f, list) else [f]):
                        if g is not None:
                            g()
                last = q2 == NST - 1 and h == 3
                drain(h, q2, o_ps, mul_eng=nc.vector if last else None,
                      dma_q=nc.scalar if last else None, last=last)
            if q2 == NST - 1:
                # last stripe's projection is the tail.  The aT[0]-side
                # matmuls only need the third head's drain, so they run
                # during the last drain's chain (also keeping the PE p-state
                # hot); the aT[1] side + stores follow.  All 8 blocks get
                # PSUM slots from the now-idle attention pools.
                slots = []
                for r in range(2):
                    tl = s_pool.tile([128, FBS], f32, name=f"pjs{r}",
                                     tag="sps")
                    slots += [tl[:, 0:FB], tl[:, FB:FBS]]
                tl = o_pool.tile([128, FBS], f32, name="pjo", tag="ops")
                slots += [tl[:, 0:FB], tl[:, FB:FBS]]
                for r in range(2):
                    slots.append(shp.tile([128, FB], f32, name=f"pjh{r}",
                                          tag="sh")[:])
                blks = list(range(q2 * 8, q2 * 8 + 8))
                # four rounds of the aT[0]-side matmuls: the extra rounds are
                # redundant recomputes (start=True overwrites with the same
                # value) that keep the PE busy through the final drain's
                # chain, so it stays at full clock for the aT[1] side
                for _ in range(4):
                    for i, blk in enumerate(blks):
                        bs = slice(blk * 128, (blk + 1) * 128)
                        nc.tensor.matmul(slots[i], aT[0][:, bs], pjt_sb[0][:],
                                         start=True, stop=False,
                                         skip_group_check=True)
                # aT[1]-side matmuls in slot-pair order with one cast per
                # half (split across the idle Scalar engine and the Vector
                # engine), stores trailing on two DMA rings
                for p in range(4):
                    for i in (2 * p, 2 * p + 1):
                        bs = slice(blks[i] * 128, (blks[i] + 1) * 128)
                        nc.tensor.matmul(slots[i], aT[1][:, bs], pjt_sb[1][:],
                                         start=False, stop=True)
                    ob = obp.tile([128, FBS], bf16, name=f"ob2_{p}",
                                  tag="ob2")
                    nc.scalar.activation(ob[:, 0:FB], slots[2 * p], AF.Copy)
                    nc.vector.tensor_copy(ob[:, FB:FBS], slots[2 * p + 1])
                    for i in (2 * p, 2 * p + 1):
                        bs = slice(blks[i] * 128, (blks[i] + 1) * 128)
                        col = slice(0, FB) if i == 2 * p else slice(FB, FBS)
                        q = nc.gpsimd if i % 2 == 0 else nc.sync
                        q.dma_start(io["out"][bs, :], ob[:, col])


def _build(cfg_key):
    from concourse import bacc, mybir, tile

    dt = mybir.dt
    nc = bacc.Bacc("TRN2", target_bir_lowering=False, debug=False,
                   num_devices=8)
    shapes = {
        "xta": ([128, 4096], dt.bfloat16),
        "xtb": ([128, 4160], dt.bfloat16),
        "mta": ([128, CA], dt.bfloat16),
        "mtb": ([128, N - CA + 16], dt.bfloat16),
        "sta": ([128, CA], dt.bfloat16),
        "stb": ([128, N - CA + 16], dt.bfloat16),
        "mt1": ([128, N + 16], dt.bfloat16),
        "st1": ([128, N + 16], dt.bfloat16),
        "wqka": ([128, 1024], dt.bfloat16), "wqkb": ([128, 1088], dt.bfloat16),
        "wv": ([128, 1024], dt.bfloat16),
        "pjtp": ([128, 1040], dt.bfloat16),
        "mcw": ([128, 8], dt.float32), "scw": ([128, 8], dt.float32),
        "qkb": ([128, 4], dt.float32), "vbrep": ([128, 256], dt.bfloat16),
    }
    io = {}
    for name, (shape, dtt) in shapes.items():
        io[name] = nc.dram_tensor(name, shape, dtt,
                                  kind="ExternalInput").ap()
    io["out"] = nc.dram_tensor("out", [N, DIM], dt.bfloat16,
                               kind="ExternalOutput").ap()
    with tile.TileContext(nc) as tc:
        _emit(tc, nc, io)
    nc.compile()
    return nc


def _get_program(cfg=None):
    key = tuple(sorted(cfg.items())) if cfg else ()
    if key not in _CACHE:
        _CACHE[key] = _build(key)
    return _CACHE[key]


# ------------------------------------------------------------------ wrapper
def kernel(_cfg=None, _want_results=False, **inputs):
    from concourse.bass_utils import run_bass_kernel_spmd

    inputs = {k: np.asarray(v, dtype=np.float32) for k, v in inputs.items()}
    nc = _get_program({})
    in_maps = [_host_prep(core, inputs) for core in range(8)]
    res = run_bass_kernel_spmd(nc, in_maps, list(range(8)))

    out = np.empty((B, N, DIM), np.float32)
    pb = inputs["proj_b"]
    for b in range(B):
        out[b] = (res.results[2 * b]["out"].astype(np.float32)
                  + res.results[2 * b + 1]["out"].astype(np.float32) + pb)
    if _want_results:
        return out, res
    return out


# revision 24
# speedup vs baseline: 1.1854x; 1.1854x over previous
"""Trainium2 Bass kernel for nn_AttentionWithVQ (B=4, N=2048, DIM=512, H=8,
depthwise-conv positional term, softmax attention, output projection).

Sharding: data-parallel over B (4 batches x 2 core-groups) and tensor-parallel
over heads (4 heads per core) -> 8 cores, fully independent per core except a
final partial-sum over the two head-groups of each batch, done on host at
gather time (the output projection contracts over heads).

Core algorithmic fusion: the score matrix
    S = 0.5*(scale * q @ k^T + scale * conv1(m) @ conv2(s)^T)
is ONE matmul over a concatenated 128-feature axis:
    S = Qp^T @ Kp,  Qp = [q*scale*0.5 ; conv1(m)*scale*0.5], Kp = [k ; conv2(s)]
which exactly fills the 128x128 PE array contraction dim.

Softmax denominators: each V stationary block is padded to 128 columns with 64
REPLICATED ones-columns shared between head pairs (block layout
[v0|1*64|v1|v2|1*64|v3]; even head reads [v|1], odd head [1|v]), so attn@V
leaves the denominator already broadcast across 64 PSUM partitions on the
opposite half from the numerator, and the numerator half matches the aT
(projection-layout) rows for BOTH parities.  The drain is then pure on-chip
dataflow: fp32 staging copy, one SBUF->SBUF partition-shift DMA, an in-place
fast-approx reciprocal, and one multiply that writes normalized output
STRAIGHT into aT (no DRAM bounce, no output shift DMAs).

Schedule: the kernel is paced by the Scalar engine's 128 exp() instructions
(~1.11us each).  Everything else is arranged around keeping that stream dense:
  - inputs are packed host-side so every DMA is one large contiguous block
    (strided HBM reads are descriptor-dominated); wave 1 carries only the
    bytes the first attention block needs, wave 2 (queue-FIFO gated) streams
    the rest behind it.  A dummy exp() preloads the ACT table during the DMAs.
  - v-projection, remaining qkv chunks, t=1 convs and the previous stripe's
    output projection are emitted as PE/DVE fillers inside the attention loop.
  - loop order stripe-outer/head-inner so each stripe's projection + output
    DMA overlaps the next stripe's attention.
"""


import sys

sys.path.insert(0, "/opt/trn_rl_repo")

import numpy as np

# ---------------------------------------------------------------- constants
B, N, DIM, HEAD, VQE_K = 4, 2048, 512, 8, 3
Dh = DIM // HEAD            # 64
HPC = HEAD // 2             # heads per core (8 cores = 4 batch * 2 groups)
P = 128
NKB = N // P                # 16 key blocks
FB = 512                    # one fp32 PSUM bank
FBS = 1024                  # attention stripe chunk (2 banks)
NST = N // FBS              # 2 q-stripes
SCALE_Q = Dh ** -0.5 * 0.5  # folds the 0.5 score scale into the q/conv1 side
VW = 384                    # v_sb block pitch: [v0|1*64|v1|v2|1*64|v3]
VOFF = (0, 64, 192, 256)    # per-head stationary column offset in a v block
CA = 1048                   # conv-input wave-1 cols (1024 + halo + WAW overlap)

_DEFAULT_CFG = {}
_CACHE = {}


# ---------------------------------------------------------------- host prep
def _host_prep(core, inp):
    """Build the per-core input arrays (sharding + layout permutations).

    Every array is packed so the device-side DMA reads one large contiguous
    block per destination region (strided HBM reads are descriptor-dominated).
    """
    import ml_dtypes

    bf16 = ml_dtypes.bfloat16
    b, g = core // 2, core % 2
    f32 = np.float32
    x, m, s = inp["x"], inp["m"], inp["s"]
    qkv_w, qkv_b = inp["qkv_w"], inp["qkv_b"]
    proj_w = inp["proj_w"]
    p1w = inp["pe1_w"].reshape(HEAD, VQE_K)
    p2w = inp["pe2_w"].reshape(HEAD, VQE_K)
    pe1_b, pe2_b = inp["pe1_b"], inp["pe2_b"]

    d = {}
    xt = np.ascontiguousarray(x[b].T).astype(bf16)       # [512, 2048]
    # xta/xtb: [128, 4*w] partition-major, c-tiles side by side.  Wave-2
    # transfers re-write a 16-column overlap of their wave-1 predecessor
    # (identical data) purely to create a write-after-write dependency that
    # orders wave 2 behind wave 1 (the tile scheduler ignores queue order).
    xta = np.empty((128, 4, 1040), bf16)
    xtb = np.empty((128, 4, 1024), bf16)
    for c in range(4):
        xta[:, c, :] = xt[c * 128:(c + 1) * 128, 0:1040]
        xtb[:, c, :] = xt[c * 128:(c + 1) * 128, 1024:2048]
    d["xta"] = xta.reshape(128, 4160)
    d["xtb"] = xtb.reshape(128, 4096)

    # m/s transposed, tile t rows = [head(2t+1) feats ; head(2t) feats]
    mt = np.empty((256, N), f32)
    st = np.empty((256, N), f32)
    mcw = np.zeros((128, 8), f32)
    scw = np.zeros((128, 8), f32)
    for t in range(2):
        h_lo, h_hi = g * 4 + 2 * t + 1, g * 4 + 2 * t
        mt[t * 128:t * 128 + 64] = m[b][:, h_lo * 64:(h_lo + 1) * 64].T
        mt[t * 128 + 64:t * 128 + 128] = m[b][:, h_hi * 64:(h_hi + 1) * 64].T
        st[t * 128:t * 128 + 64] = s[b][:, h_lo * 64:(h_lo + 1) * 64].T
        st[t * 128 + 64:t * 128 + 128] = s[b][:, h_hi * 64:(h_hi + 1) * 64].T
        for p in range(128):
            h = g * 4 + 2 * t + (1 if p < 64 else 0)
            mcw[p, 4 * t:4 * t + 3] = p1w[h] * SCALE_Q
            scw[p, 4 * t:4 * t + 3] = p2w[h]
            mcw[p, 4 * t + 3] = pe1_b[h] * SCALE_Q
            scw[p, 4 * t + 3] = pe2_b[h]
    mtc, stc = mt.astype(bf16), st.astype(bf16)
    # conv inputs live in one [128, 4096] tile per source (t0 | t1); the
    # wave-2 pieces overlap their predecessor by 16 cols (WAW chaining)
    d["mta"] = np.ascontiguousarray(mtc[0:128, 0:CA])
    d["mtb"] = np.ascontiguousarray(mtc[0:128, CA - 16:N])
    d["sta"] = np.ascontiguousarray(stc[0:128, 0:CA])
    d["stb"] = np.ascontiguousarray(stc[0:128, CA - 16:N])
    d["mt1"] = np.ascontiguousarray(
        np.concatenate([mtc[0:128, N - 16:N], mtc[128:256, :]], axis=1))
    d["st1"] = np.ascontiguousarray(
        np.concatenate([stc[0:128, N - 16:N], stc[128:256, :]], axis=1))
    d["mcw"], d["scw"] = mcw, scw

    # q/k projection weights: chunk ch=(t, q|k) = [even-head rows; odd-head rows]
    wqk_f = np.empty((512, DIM), f32)
    qkb = np.zeros((128, 4), f32)
    for t in range(2):
        for j in range(2):  # 0=q, 1=k
            ch = 2 * t + j
            h_e, h_o = g * 4 + 2 * t, g * 4 + 2 * t + 1
            base = j * DIM
            wqk_f[ch * 128:ch * 128 + 64] = qkv_w[base + h_e * 64:base + (h_e + 1) * 64]
            wqk_f[ch * 128 + 64:(ch + 1) * 128] = qkv_w[base + h_o * 64:base + (h_o + 1) * 64]
            qkb[0:64, ch] = qkv_b[base + h_e * 64:base + (h_e + 1) * 64]
            qkb[64:128, ch] = qkv_b[base + h_o * 64:base + (h_o + 1) * 64]
            if j == 0:
                wqk_f[ch * 128:(ch + 1) * 128] *= SCALE_Q
                qkb[:, ch] *= SCALE_Q
    wqk_t = np.ascontiguousarray(wqk_f.T).astype(bf16)   # [c=512, f=512]
    wqka = np.empty((128, 4, 272), bf16)                 # ch0/ch1 + pad
    wqkb = np.empty((128, 4, 256), bf16)                 # ch2/ch3
    for c in range(4):
        wqka[:, c, :] = wqk_t[c * 128:(c + 1) * 128, 0:272]
        wqkb[:, c, :] = wqk_t[c * 128:(c + 1) * 128, 256:512]
    d["wqka"] = wqka.reshape(128, 1088)
    d["wqkb"] = wqkb.reshape(128, 1024)
    d["qkb"] = qkb

    wv_t = np.ascontiguousarray(
        qkv_w[2 * DIM + g * 256:2 * DIM + (g + 1) * 256].T).astype(bf16)
    wvp = np.empty((128, 4, 256), bf16)
    for c in range(4):
        wvp[:, c, :] = wv_t[c * 128:(c + 1) * 128, :]
    d["wv"] = wvp.reshape(128, 1024)
    # v bias replicated along partitions: column order matches wv columns
    vb = qkv_b[2 * DIM + g * 256:2 * DIM + (g + 1) * 256]
    d["vbrep"] = np.broadcast_to(vb, (128, 256)).astype(bf16).copy()

    # proj rows in aT partition order: aT tile t partition p -> head
    # 2t+(p>=64), d=p%64.  Packed partition-major behind a 16-col duplicate
    # of the wqk tail so the transfer WAW-chains into the wqk tile region.
    pjt = np.empty((256, DIM), f32)
    for t in range(2):
        for p in range(128):
            h_l = 2 * t + (1 if p >= 64 else 0)
            h = g * 4 + h_l
            pjt[t * 128 + p] = proj_w[:, h * 64 + (p % 64)]
    pjtc = pjt.astype(bf16)
    pjtp = np.empty((128, 1040), bf16)
    pjtp[:, 0:16] = wqk_t[384:512, 496:512]
    for f in range(2):
        pjtp[:, 16 + f * 512:16 + (f + 1) * 512] = pjtc[f * 128:(f + 1) * 128]
    d["pjtp"] = pjtp
    return d


# ------------------------------------------------------------- device build
def _emit(tc, nc, io):
    from contextlib import ExitStack

    from concourse import mybir

    dt = mybir.dt
    f32 = dt.float32
    bf16 = dt.bfloat16
    AF = mybir.ActivationFunctionType
    ALU = mybir.AluOpType

    with ExitStack() as ctx:
        persist = ctx.enter_context(tc.tile_pool(name="persist", bufs=1))
        xtp = ctx.enter_context(tc.tile_pool(name="xtp", bufs=1))
        convp = ctx.enter_context(tc.tile_pool(name="convp", bufs=2))
        convyp = ctx.enter_context(tc.tile_pool(name="convyp", bufs=2))
        # PSUM: s_pool 2x2 banks, o_pool 1x2 banks, shp 2x1 bank = 8 banks
        s_pool = ctx.enter_context(
            tc.tile_pool(name="s_pool", bufs=2, space="PSUM"))
        o_pool = ctx.enter_context(
            tc.tile_pool(name="o_pool", bufs=1, space="PSUM"))
        shp = ctx.enter_context(tc.tile_pool(name="shp", bufs=2, space="PSUM"))
        esb = ctx.enter_context(tc.tile_pool(name="esb", bufs=8))
        stgp = ctx.enter_context(tc.tile_pool(name="stgp", bufs=2))
        bcp = ctx.enter_context(tc.tile_pool(name="bcp", bufs=2))
        obp = ctx.enter_context(tc.tile_pool(name="obp", bufs=3))

        # ---- persistent tiles (pjt lives in the wqk tile's tail so its
        # wave-2 transfer can WAW-chain behind the wqkb transfer)
        wqk_all = persist.tile([128, 3072], bf16, name="wqk", tag="wqk")
        wqk_sb = [wqk_all[:, c * 512:(c + 1) * 512] for c in range(4)]
        pjt_sb = [wqk_all[:, 2048 + f * 512:2048 + (f + 1) * 512]
                  for f in range(2)]
        wv_all = persist.tile([128, 1024], bf16, name="wv", tag="wv")
        wv_sb = [wv_all[:, c * 256:(c + 1) * 256] for c in range(4)]
        mcw_sb = persist.tile([128, 8], f32, name="mcw", tag="mcw")
        scw_sb = persist.tile([128, 8], f32, name="scw", tag="scw")
        qkb_sb = persist.tile([128, 4], f32, name="qkb", tag="qkb")
        vbr_sb = persist.tile([128, 256], bf16, name="vbrep", tag="vbrep")
        QP = [persist.tile([128, N], bf16, name=f"QP{h}", tag=f"QP{h}")
              for h in range(HPC)]
        KP = [persist.tile([128, N], bf16, name=f"KP{h}", tag=f"KP{h}")
              for h in range(HPC)]
        # one big V tile: 16 blocks at VW pitch [v0|1*64|v1|v2|1*64|v3], with
        # 64 cols of tail padding so the strided odd-head write AP stays in
        # bounds.  The ones-columns sit at 64 + k*192 for k=0..31, written by
        # ONE strided memset.
        v_big = persist.tile([128, NKB * VW + 128], bf16, name="vbig",
                             tag="vbig")
        aT = [persist.tile([128, N], bf16, name=f"aT{t}", tag=f"aT{t}")
              for t in range(2)]
        xt_all = xtp.tile([128, 4 * N], bf16, name="xt", tag="xt")
        xt_sb = [xt_all[:, c * N:(c + 1) * N] for c in range(4)]

        # conv inputs: one [128, 4096] tile per source, t0 | t1 halves
        cinS = {src: convp.tile([128, 2 * N], bf16, name=f"ci_{src}",
                                tag="cin") for src in ("st", "mt")}
        cin0 = {src: cinS[src][:, 0:N] for src in ("st", "mt")}
        cin1 = {src: cinS[src][:, N:2 * N] for src in ("st", "mt")}

        # ---- input DMAs.  Wave 1 = only what the first attention block
        # needs, every transfer one large contiguous HBM block.  The ACT
        # exp-table preload rides the scalar queue behind the tiny tiles.
        nc.scalar.dma_start(mcw_sb[:], io["mcw"][:, :])
        nc.scalar.dma_start(scw_sb[:], io["scw"][:, :])
        nc.scalar.dma_start(qkb_sb[:], io["qkb"][:, :])
        dum = persist.tile([1, 16], f32, name="dum", tag="dum")
        nc.vector.memset(dum[:], 0.0)
        nc.scalar.activation(dum[:], dum[:], AF.Exp)
        # warm-up matmuls: ~4us of PE activity trips the HAM clock gate to
        # full rate before the projection chunks need it
        warm = persist.tile([128, 512], bf16, name="warm", tag="warm")
        nc.vector.memset(warm[:], 0.25)
        wps = shp.tile([128, FB], f32, name="wps", tag="sh")
        for _ in range(7):
            nc.tensor.matmul(wps[:], warm[:, 0:128], warm[:],
                             start=True, stop=True, skip_group_check=True)
        # the ones-columns of every v block, one strided memset
        nc.vector.memset(
            v_big[:, 64:64 + 32 * 192].rearrange("p (s r) -> p s r",
                                                 r=192)[:, :, 0:64], 1.0)

        xtv = xt_all.rearrange("p (c n) -> p c n", c=4)
        nc.sync.dma_start(xtv[:, :, 0:1040],
                          io["xta"].rearrange("p (c n) -> p c n", c=4))
        nc.sync.dma_start(cin0["mt"][:, 0:CA], io["mta"][:, :])
        nc.sync.dma_start(cin0["st"][:, 0:CA], io["sta"][:, :])
        nc.gpsimd.dma_start(
            wqk_all[:, 0:2048].rearrange("p (c f) -> p c f", c=4)[:, :, 0:272],
            io["wqka"].rearrange("p (c f) -> p c f", c=4))
        nc.gpsimd.dma_start(
            wv_all.rearrange("p (c f) -> p c f", c=4),
            io["wv"].rearrange("p (c f) -> p c f", c=4))
        nc.gpsimd.dma_start(vbr_sb[:], io["vbrep"][:, :])

        # ---- helpers -----------------------------------------------------
        convy = {}

        def conv_ops(src, wv_, dst, t, xin, c0=0, c1=N):
            """Depthwise 3-tap conv for columns [c0,c1) of tile t of m/s.
            Column-ranged so the first chunk (which gates the first attention
            iteration) finishes early."""
            key = (src, t)
            if key not in convy:
                convy[key] = convyp.tile([128, N], bf16, name=f"cy_{src}{t}",
                                         tag="cy")
            y = convy[key]
            w0, w1, w2, cb = (wv_[:, 4 * t + k:4 * t + k + 1] for k in range(4))
            lo = max(c0, 1)
            hi = min(c1, N - 1)
            nc.vector.tensor_scalar(y[:, c0:c1], xin[:, c0:c1], w1, cb,
                                    ALU.mult, ALU.add)
            nc.vector.scalar_tensor_tensor(
                y[:, lo:c1], xin[:, lo - 1:c1 - 1], w0, y[:, lo:c1],
                ALU.mult, ALU.add)
            nc.vector.scalar_tensor_tensor(
                y[:, c0:hi], xin[:, c0 + 1:hi + 1], w2, y[:, c0:hi],
                ALU.mult, ALU.add)
            nc.vector.tensor_copy(dst[2 * t + 1][0:64, c0:c1], y[0:64, c0:c1])
            nc.vector.tensor_copy(dst[2 * t][64:128, c0:c1], y[64:128, c0:c1])

        def qkv_chunk(ch, qs, pool, tag, width):
            """q/k projection chunk ch over q-columns qs (width cols)."""
            for step in qkv_chunk_steps(ch, qs, pool, tag, width):
                step()

        def qkv_chunk_steps(ch, qs, pool, tag, width):
            """Same, but as a list of single-matmul emission steps so the
            chunk can be spread across attention iterations."""
            t, j = ch // 2, ch % 2
            dst = QP if j == 0 else KP
            nh = width // FB
            state = {}

            def mk(ih, c):
                def step():
                    if "ps" not in state:
                        state["ps"] = pool.tile([128, width], f32,
                                                name="psqk", tag=tag)
                    ps = state["ps"]
                    nc.tensor.matmul(
                        ps[:, ih * FB:(ih + 1) * FB],
                        wqk_sb[c][:, ch * 128:(ch + 1) * 128],
                        xt_sb[c][:, qs.start + ih * FB:qs.start + (ih + 1) * FB],
                        start=(c == 0), stop=(c == 3))
                    if ih == nh - 1 and c == 3:
                        nc.vector.tensor_scalar_add(
                            dst[2 * t][0:64, qs], ps[0:64, :],
                            qkb_sb[0:64, ch:ch + 1])
                        nc.vector.tensor_scalar_add(
                            dst[2 * t + 1][64:128, qs], ps[64:128, :],
                            qkb_sb[64:128, ch:ch + 1])
                return step

            return [mk(ih, c) for ih in range(nh) for c in range(4)]

        def v_block(blk):
            """v projection for key-block blk + bias into the [v|1]/[1|v]
            slots (even heads at cols {0,192}, odd heads at {128,320})."""
            bs = slice(blk * 128, (blk + 1) * 128)
            ps = shp.tile([128, 512], f32, name="psv", tag="sh")
            for c in range(4):
                nc.tensor.matmul(ps[:, 0:256], xt_sb[c][:, bs], wv_sb[c][:],
                                 start=(c == 0), stop=(c == 3))
            ps4 = ps[:, 0:256].rearrange("p (s r) -> p s r", s=2)
            vb4 = vbr_sb.rearrange("p (s r) -> p s r", s=2)
            dste = v_big[:, blk * VW:blk * VW + 384].rearrange(
                "p (s r) -> p s r", r=192)[:, :, 0:64]
            dsto = v_big[:, blk * VW + 128:blk * VW + 512].rearrange(
                "p (s r) -> p s r", r=192)[:, :, 0:64]
            nc.vector.scalar_tensor_tensor(
                dste, ps4[:, :, 0:64], 1.0, vb4[:, :, 0:64],
                ALU.mult, ALU.add)
            nc.vector.scalar_tensor_tensor(
                dsto, ps4[:, :, 64:128], 1.0, vb4[:, :, 64:128],
                ALU.mult, ALU.add)

        def drain(h, q2, o_ps, mul_eng=None, dma_q=None, last=False):
            """Normalize o_ps by the softmax denominators into aT.

            One fp32 staging copy frees o_ps; the denominator (already
            replicated across 64 partitions by the ones-columns of V) is
            partition-shifted to the numerator's half by a single SBUF->SBUF
            DMA, approx-reciprocal'd in place, and multiplied straight into
            aT (both parities lane-aligned)."""
            t, odd = h // 2, h % 2
            if mul_eng is None:
                mul_eng = nc.gpsimd
            dq = dma_q if dma_q is not None else nc.sync
            rows = slice(64, 128) if odd else slice(0, 64)
            drows = slice(0, 64) if odd else slice(64, 128)
            row = h * NST + q2
            stg = stgp.tile([128, FBS], f32, name=f"stg{row}", tag="stg")
            bc = bcp.tile([128, 2 * FBS], f32, name=f"bc{row}", tag="bc")
            cs = slice(q2 * FBS, (q2 + 1) * FBS)
            # single PSUM read frees o_ps for the next accumulation.  The
            # custom-DVE reciprocal only works at partition base 0, so the
            # partition-shift DMA runs before it (even heads: den lives at
            # 64:128) or after it (odd heads: den already at 0:64).  For the
            # last (odd) drain, the reciprocal reads the denominator straight
            # from PSUM so it starts before the staging copy finishes.
            if last and odd:
                # no staging at all: reciprocal straight off PSUM, bf16-cast
                # partition-shift on the SWDGE ring, multiply off PSUM
                bcl = stgp.tile([128, FBS], bf16, name="bcl", tag="stg")
                nc.vector.reciprocal_approx_fast(bc[0:64, 0:FBS],
                                                 o_ps[0:64, :])
                nc.gpsimd.dma_start(bcl[64:128, :], bc[0:64, 0:FBS])
                mul_eng.tensor_mul(aT[t][rows, cs], o_ps[rows, :],
                                   bcl[rows, :])
                return
            if odd:
                nc.vector.tensor_copy(stg[:], o_ps[:])
                nc.vector.reciprocal_approx_fast(bc[0:64, 0:FBS],
                                                 stg[0:64, :])
                dq.dma_start(bc[64:128, FBS:2 * FBS], bc[0:64, 0:FBS])
            else:
                nc.vector.tensor_copy(stg[:], o_ps[:])
                dq.dma_start(bc[0:64, 0:FBS], stg[64:128, :])
                nc.vector.reciprocal_approx_fast(bc[0:64, FBS:2 * FBS],
                                                 bc[0:64, 0:FBS])
            mul_eng.tensor_mul(aT[t][rows, cs], stg[rows, :],
                               bc[rows, FBS:2 * FBS])

        def proj_block_steps(blk):
            """Output projection for 128 q rows + bf16 store, as 2 steps."""
            bs = slice(blk * 128, (blk + 1) * 128)
            state = {}

            def s0():
                state["pj"] = shp.tile([128, FB], f32, name="pj", tag="sh")
                nc.tensor.matmul(state["pj"][:], aT[0][:, bs], pjt_sb[0][:],
                                 start=True, stop=False)

            def s1():
                pj = state["pj"]
                nc.tensor.matmul(pj[:], aT[1][:, bs], pjt_sb[1][:],
                                 start=False, stop=True)
                ob = obp.tile([128, FB], bf16, name="ob", tag="ob")
                nc.vector.tensor_copy(ob[:], pj[:])
                nc.sync.dma_start(io["out"][bs, :], ob[:])

            return [s0, s1]

        def proj_block(blk):
            for s in proj_block_steps(blk):
                s()

        # ---- prologue compute: t=0 convs (first chunks early, distinct
        # PSUM slots per qkv chunk so none waits on another's bias), the qkv
        # chunks head 0 stripe 0 needs; everything else is emitted as
        # fillers inside the attention loop.
        conv_ops("mt", mcw_sb, QP, 0, cin0["mt"], 0, 1024)
        conv_ops("st", scw_sb, KP, 0, cin0["st"], 0, 256)
        qkv_chunk(1, slice(0, 512), s_pool, "sps", 512)      # k(t0) 0:512
        qkv_chunk(1, slice(512, 1024), o_pool, "ops", 512)   # k(t0) 512:1024
        qkv_chunk(0, slice(0, 512), s_pool, "sps", 512)      # q(t0) 0:512
        qkv_chunk(0, slice(512, 1024), shp, "sh", 512)       # q(t0) 512:1024
        conv_ops("st", scw_sb, KP, 0, cin0["st"], 256, 1024)

        # ---- wave 2 DMAs.  Each transfer re-writes a 16-col overlap of its
        # wave-1 (or earlier wave-2) predecessor with identical data: the
        # write-after-write dependency orders it behind the predecessor's
        # completion (the tile scheduler reorders queues, so FIFO gating
        # alone cannot hold wave 2 back).
        nc.gpsimd.dma_start(
            xt_all.rearrange("p (c n) -> p c n", c=4)[:, :, 1024:2048],
            io["xtb"].rearrange("p (c n) -> p c n", c=4))
        nc.gpsimd.dma_start(cinS["st"][:, CA - 16:N], io["stb"][:, :])
        nc.gpsimd.dma_start(cinS["st"][:, N - 16:2 * N], io["st1"][:, :])
        nc.gpsimd.dma_start(cinS["mt"][:, CA - 16:N], io["mtb"][:, :])
        nc.gpsimd.dma_start(cinS["mt"][:, N - 16:2 * N], io["mt1"][:, :])
        nc.gpsimd.dma_start(
            wqk_all[:, 0:2048].rearrange("p (c f) -> p c f",
                                         c=4)[:, :, 256:512],
            io["wqkb"].rearrange("p (c f) -> p c f", c=4))
        nc.gpsimd.dma_start(wqk_all[:, 2032:3072], io["pjtp"][:, :])
        conv_ops("st", scw_sb, KP, 0, cin0["st"], 1024, 2048)

        # ---- attention: stripe-outer, head-inner, exp-paced.  Fillers are
        # single-matmul-sized emission steps, one consumed per nk iteration.
        def fillers_for(h, q2):
            # an entry may be a list of sub-steps (all emitted in one slot)
            fl = []
            if q2 == 0 and h == 0:
                # v blocks ride just-in-time: with the skewed loop, aV(k) is
                # emitted at iter k+1, so v(k) sits at slot k
                for blk in range(NKB):
                    fl.append(lambda b_=blk: v_block(b_))
                # k(t0) cols 1024:2048 in two 512-chunks; their xt columns
                # arrive with the second DMA wave
                qa = qkv_chunk_steps(1, slice(1024, 1536), shp, "sh", 512)
                qb = qkv_chunk_steps(1, slice(1536, 2048), shp, "sh", 512)
                for i, s in enumerate(qa):
                    fl[2 + i] = [fl[2 + i], s]
                for i, s in enumerate(qb):
                    fl[7 + i] = [fl[7 + i], s]
                fl[15] = [fl[15],
                          lambda: conv_ops("st", scw_sb, KP, 1, cin1["st"])]
            elif q2 == 0 and h == 1:
                # k(t1)+q(t1) stripe-0 columns and the t=1 q-side conv
                # (stripe-0 half) — all due by h2 iter 0
                for ch, qb in ((3, 0), (2, 0), (2, 1)):
                    fl += qkv_chunk_steps(ch, slice(qb * 512, (qb + 1) * 512),
                                          shp, "sh", 512)
                fl.insert(6, lambda: conv_ops("mt", mcw_sb, QP, 1,
                                              cin1["mt"], 0, 1024))
                fl.append(lambda: conv_ops("mt", mcw_sb, QP, 0, cin0["mt"],
                                           1024, N))
            elif q2 == 0 and h == 2:
                # k(t1) remaining columns (due by h2 iters 4/8/12) + the
                # stripe-1 half of the t=1 q-side conv
                for ch, qb in ((3, 1), (3, 2), (3, 3)):
                    fl += qkv_chunk_steps(ch, slice(qb * 512, (qb + 1) * 512),
                                          shp, "sh", 512)
                fl.append(lambda: conv_ops("mt", mcw_sb, QP, 1, cin1["mt"],
                                           1024, N))
            elif q2 == 0 and h == 3:
                # q(t0)/q(t1) stripe-1 columns (due by stripe 1)
                for ch, qb in ((0, 2), (0, 3), (2, 2), (2, 3)):
                    fl += qkv_chunk_steps(ch, slice(qb * 512, (qb + 1) * 512),
                                          shp, "sh", 512)
            elif q2 == 1 and h == 0:
                # previous stripe's projection; pad the first slots so the
                # PE never head-of-line blocks on the preceding drain
                fl += [None] * 6
                fl += proj_block_steps(0)
                fl += proj_block_steps(1)
            elif q2 == 1 and h == 2:
                for blk in range(2, 6):
                    fl += proj_block_steps(blk)
            elif q2 == 1 and h == 1:
                fl += proj_block_steps(6)
                fl += proj_block_steps(7)
            return fl

        for q2 in range(NST):
            for h in ((0, 1, 2, 3) if q2 == 0 else (0, 2, 1, 3)):
                voff = VOFF[h]
                cs0 = q2 * FBS
                fl = fillers_for(h, q2)
                o_ps = o_pool.tile([128, FBS], f32, name=f"o{h}_{q2}",
                                   tag="ops")
                # skewed pipeline: scores run one iteration ahead of
                # exp/attnV, so the first attnV's wait on the previous
                # head's staging copy hides behind already-queued scores
                def exp_av(nk, s_prev):
                    e = esb.tile([128, FBS], bf16, name="e", tag="e")
                    nc.scalar.activation(e[:], s_prev[:], AF.Exp)
                    vst = v_big[:, nk * VW + voff:nk * VW + voff + 128]
                    for ih in range(2):
                        nc.tensor.matmul(
                            o_ps[:, ih * FB:(ih + 1) * FB], vst,
                            e[:, ih * FB:(ih + 1) * FB],
                            start=(nk == 0), stop=(nk == NKB - 1))

                s_prev = None
                for nk in range(NKB):
                    if fl:
                        f = fl.pop(0)
                        for g in (f if isinstance(f, list) else [f]):
                            if g is not None:
                                g()
                    ks = slice(nk * 128, (nk + 1) * 128)
                    s_ps = s_pool.tile([128, FBS], f32, name="sps", tag="sps")
                    for ih in range(2):
                        nc.tensor.matmul(
                            s_ps[:, ih * FB:(ih + 1) * FB], KP[h][:, ks],
                            QP[h][:, cs0 + ih * FB:cs0 + (ih + 1) * FB],
                            start=True, stop=True)
                    if s_prev is not None:
                        exp_av(nk - 1, s_prev)
                    s_prev = s_ps
                exp_av(NKB - 1, s_prev)
                for f in fl:
                    for g in (f if isinstance(f, list) else [f]):
                        if g is not None:
                            g()
                last = q2 == NST - 1 and h == 3
                drain(h, q2, o_ps, mul_eng=nc.vector if last else None,
                      dma_q=nc.scalar if last else None, last=last)
            if q2 == NST - 1:
                # last stripe's projection is the tail.  The aT[0]-side
                # matmuls only need the third head's drain, so they run
                # during the last drain's chain (also keeping the PE p-state
                # hot); the aT[1] side + stores follow.  All 8 blocks get
                # PSUM slots from the now-idle attention pools.
                slots = []
                for r in range(2):
                    tl = s_pool.tile([128, FBS], f32, name=f"pjs{r}",
                                     tag="sps")
                    slots += [tl[:, 0:FB], tl[:, FB:FBS]]
                tl = o_pool.tile([128, FBS], f32, name="pjo", tag="ops")
                slots += [tl[:, 0:FB], tl[:, FB:FBS]]
                for r in range(2):
                    slots.append(shp.tile([128, FB], f32, name=f"pjh{r}",
                                          tag="sh")[:])
                blks = list(range(q2 * 8, q2 * 8 + 8))
                # four rounds of the aT[0]-side matmuls: the extra rounds are
                # redundant recomputes (start=True overwrites with the same
                # value) that keep the PE busy through the final drain's
                # chain, so it stays at full clock for the aT[1] side
                for _ in range(4):
                    for i, blk in enumerate(blks):
                        bs = slice(blk * 128, (blk + 1) * 128)
                        nc.tensor.matmul(slots[i], aT[0][:, bs], pjt_sb[0][:],
                                         start=True, stop=False,
                                         skip_group_check=True)
                # aT[1]-side matmuls in slot-pair order with one cast per
                # half (split across the idle Scalar engine and the Vector
                # engine), stores trailing on two DMA rings
                for p in range(4):
                    for i in (2 * p, 2 * p + 1):
                        bs = slice(blks[i] * 128, (blks[i] + 1) * 128)
                        nc.tensor.matmul(slots[i], aT[1][:, bs], pjt_sb[1][:],
                                         start=False, stop=True)
                    ob = obp.tile([128, FBS], bf16, name=f"ob2_{p}",
                                  tag="ob2")
                    nc.scalar.activation(ob[:, 0:FB], slots[2 * p], AF.Copy)
                    nc.vector.tensor_copy(ob[:, FB:FBS], slots[2 * p + 1])
                    for i in (2 * p, 2 * p + 1):
                        bs = slice(blks[i] * 128, (blks[i] + 1) * 128)
                        col = slice(0, FB) if i == 2 * p else slice(FB, FBS)
                        q = nc.gpsimd if i % 2 == 0 else nc.sync
                        q.dma_start(io["out"][bs, :], ob[:, col])


def _build(cfg_key):
    from concourse import bacc, mybir, tile

    dt = mybir.dt
    nc = bacc.Bacc("TRN2", target_bir_lowering=False, debug=False,
                   num_devices=8)
    shapes = {
        "xta": ([128, 4160], dt.bfloat16),
        "xtb": ([128, 4096], dt.bfloat16),
        "mta": ([128, CA], dt.bfloat16),
        "mtb": ([128, N - CA + 16], dt.bfloat16),
        "sta": ([128, CA], dt.bfloat16),
        "stb": ([128, N - CA + 16], dt.bfloat16),
        "mt1": ([128, N + 16], dt.bfloat16),
        "st1": ([128, N + 16], dt.bfloat16),
        "wqka": ([128, 1088], dt.bfloat16), "wqkb": ([128, 1024], dt.bfloat16),
        "wv": ([128, 1024], dt.bfloat16),
        "pjtp": ([128, 1040], dt.bfloat16),
        "mcw": ([128, 8], dt.float32), "scw": ([128, 8], dt.float32),
        "qkb": ([128, 4], dt.float32), "vbrep": ([128, 256], dt.bfloat16),
    }
    io = {}
    for name, (shape, dtt) in shapes.items():
        io[name] = nc.dram_tensor(name, shape, dtt,
                                  kind="ExternalInput").ap()
    io["out"] = nc.dram_tensor("out", [N, DIM], dt.bfloat16,
                               kind="ExternalOutput").ap()
    with tile.TileContext(nc) as tc:
        _emit(tc, nc, io)
    nc.compile()
    return nc


def _get_program(cfg=None):
    key = tuple(sorted(cfg.items())) if cfg else ()
    if key not in _CACHE:
        _CACHE[key] = _build(key)
    return _CACHE[key]


# ------------------------------------------------------------------ wrapper
def kernel(_cfg=None, _want_results=False, **inputs):
    from concourse.bass_utils import run_bass_kernel_spmd

    inputs = {k: np.asarray(v, dtype=np.float32) for k, v in inputs.items()}
    nc = _get_program({})
    in_maps = [_host_prep(core, inputs) for core in range(8)]
    res = run_bass_kernel_spmd(nc, in_maps, list(range(8)))

    out = np.empty((B, N, DIM), np.float32)
    pb = inputs["proj_b"]
    for b in range(B):
        out[b] = (res.results[2 * b]["out"].astype(np.float32)
                  + res.results[2 * b + 1]["out"].astype(np.float32) + pb)
    if _want_results:
        return out, res
    return out
